# revision 8
# baseline (speedup 1.0000x reference)
# Dissipation network Bass kernel for TRN2 (bf16 matmuls, PG-fused block-diag).
#
# Layout: each "super-tile" (ST) covers 2*F batch rows = 2 partition groups
# (PG0 -> partitions 0:50 / 0:16, PG1 -> 64:114 / 64:80) x F free columns.
# Activations are stored transposed: [features, batch_cols].
# Each weight is packed BLOCK-DIAGONALLY: PG0 copy at rows rb:rb+k, cols
# 0:m and PG1 copy at rows 64+rb:.., cols 64:64+m, so ONE matmul
# instruction computes both partition groups.
# PSUM tiles for each super-tile are allocated UPFRONT in dependency
# order (x1, xs1, x2, s1, xs2, x3, s2, xs3, s3, out) so the pool's
# round-robin rotation (bufs=3) pairs each tile with one that releases
# early - avoiding false WAR serialization of the L3/L4 matmuls behind
# the slow xs gate chains.
# The final [B,1] output is NOT softplus'd per super-tile: the two psum
# rows are copied to SBUF, DMA-packed into a [2*nst, F] tile and a
# single softplus runs at the end (saves ~60us of ACT time).
# Input is converted to bf16 on the host: halves input DMA, transposes
# run at 1 cycle/row, and the transpose psum tile fits one PSUM bank.
# Softplus = Exp (with bias) then Ln(t + 1); both pinned to the
# natural_log_exp_and_others ACT table set (single table load).
import numpy as np
import ml_dtypes
import concourse.bass as bass
from concourse import bacc
import concourse.hw_specs as hw_specs
import concourse.bacc as bacc_mod
import concourse.mybir as mybir
import concourse.tile as tile

dt = mybir.dt
AF = mybir.ActivationFunctionType
ALU = mybir.AluOpType

_orig_get_tables = hw_specs.get_activation_tables


def _pinned_tables(arch):
    t = _orig_get_tables(arch)
    out = {}
    for name, fns in t.items():
        if name != "natural_log_exp_and_others":
            fns = fns - {AF.Exp, AF.Ln}
        out[name] = fns
    return out


bacc_mod.get_activation_tables = _pinned_tables

D, H = 16, 50
F = 1024            # free columns per PG block
STB = 2 * F         # rows per super-tile
NCHUNK = F // 512   # 512-col matmul chunks per F
NA = F // 128       # 128-col transpose chunks per F

W_SPECS = [
    ("W_xl1", D, H), ("W_xin", D, H), ("W_clinm", D, D), ("W_clin", D, H),
    ("W_xl2", H, H), ("W_cp1m", H, H), ("W_cl1m", H, D), ("W_xp1", H, H),
    ("W_cp1", H, H), ("W_cl1", D, H),
    ("W_xl3", H, H), ("W_cp2m", H, H), ("W_cl2m", H, D), ("W_xp2", H, H),
    ("W_cp2", H, H), ("W_cl2", D, H),
    ("W_xlo", H, 1), ("W_cpom", H, H), ("W_clom", H, D),
    ("W_cpo", H, 1), ("W_clo", D, 1),
]
W_KM = {n: (k, m) for n, k, m in W_SPECS}
X0_WEIGHTS = {"W_xl1", "W_xin", "W_clinm"}
W_OFF = {}
_off = 0
for _n, _k, _m in W_SPECS:
    W_OFF[_n] = _off
    _off += 64 + _m     # block-diag span: PG0 cols 0:m, PG1 cols 64:64+m
NW = _off


def w_spans(name):
    """(K_span, M_span) for the fused block-diagonal matmul."""
    k, m = W_KM[name]
    rb = 32 if name in X0_WEIGHTS else 0
    return 64 + rb + k, 64 + m


B_SPECS = ["b_xl1", "b_xin", "b_clinm", "b_xl2", "b_cp1m", "b_cl1m", "b_xp1",
           "b_xl3", "b_cp2m", "b_cl2m", "b_xp2", "b_xlo", "b_cpom", "b_clom"]
B_COL = {n: i for i, n in enumerate(B_SPECS)}
NB = len(B_SPECS) + 1
BXLO_ALL = NB - 1   # b_xlo replicated to all partitions (final staged softplus)


def pack_weights(inputs):
    wpack = np.zeros((128, NW), dtype=ml_dtypes.bfloat16)
    for n, k, m in W_SPECS:
        wt = np.asarray(inputs[n]).astype(np.float32).T  # [K, M]
        assert wt.shape == (k, m), (n, wt.shape)
        wb = wt.astype(ml_dtypes.bfloat16)
        rb = 32 if n in X0_WEIGHTS else 0
        o = W_OFF[n]
        wpack[rb:rb + k, o:o + m] = wb
        wpack[64 + rb:64 + rb + k, o + 64:o + 64 + m] = wb
    bpack = np.zeros((128, NB), dtype=np.float32)
    for n in B_SPECS:
        b = np.asarray(inputs[n]).astype(np.float32)
        c = B_COL[n]
        bpack[0:len(b), c] = b
        bpack[64:64 + len(b), c] = b
    bpack[:, BXLO_ALL] = float(np.asarray(inputs["b_xlo"]).astype(np.float32)[0])
    ident = np.eye(128, dtype=ml_dtypes.bfloat16)
    return wpack, bpack, ident


def build_program(n_rows):
    assert n_rows % STB == 0
    nst = n_rows // STB
    assert nst <= 64
    nc = bacc.Bacc("TRN2", target_bir_lowering=False, debug=False,
                   enable_asserts=False)
    inp_d = nc.dram_tensor("input", [n_rows, 32], dt.bfloat16, kind="ExternalInput")
    w_d = nc.dram_tensor("wpack", [128, NW], dt.bfloat16, kind="ExternalInput")
    b_d = nc.dram_tensor("bpack", [128, NB], dt.float32, kind="ExternalInput")
    c_d = nc.dram_tensor("ident", [128, 128], dt.bfloat16, kind="ExternalInput")
    out_d = nc.dram_tensor("out", [n_rows, 1], dt.float32, kind="ExternalOutput")

    with tile.TileContext(nc) as tc:
        with tc.tile_pool(name="const", bufs=1) as cpool, \
             tc.tile_pool(name="inp", bufs=6) as inpool, \
             tc.tile_pool(name="x0p", bufs=6) as x0pool, \
             tc.tile_pool(name="mh", bufs=12) as mhpool, \
             tc.tile_pool(name="g", bufs=6) as gpool, \
             tc.tile_pool(name="stg", bufs=12) as stgpool, \
             tc.tile_pool(name="axs", bufs=12) as xspool, \
             tc.tile_pool(name="ax", bufs=6) as xpool, \
             tc.tile_pool(name="zc", bufs=4) as zcpool, \
             tc.tile_pool(name="ps", bufs=3, space="PSUM") as ps, \
             tc.tile_pool(name="psb", bufs=2, space="PSUM") as psb:

            wt = cpool.tile([128, NW], dt.bfloat16, tag="wt")
            nc.sync.dma_start(out=wt[:], in_=w_d.ap())
            bt = cpool.tile([128, NB], dt.float32, tag="bt")
            nc.sync.dma_start(out=bt[:], in_=b_d.ap())
            ct = cpool.tile([128, 128], dt.bfloat16, tag="ct")
            nc.sync.dma_start(out=ct[:], in_=c_d.ap())
            # packed final-layer pre-activations: partition 2*st+pg holds
            # z_out for that PG's F batch rows
            zout = cpool.tile([2 * nst, F], dt.float32, tag="zout")

            # Zero the in_t staging buffers once: fused matmuls read the
            # full partition span (junk rows hit zero weights; must not
            # be NaN).
            for _ in range(6):
                z = inpool.tile([128, NA * 112], dt.bfloat16, tag="int")
                nc.vector.memset(z[:], 0.0)

            def alloc_st(st):
                A = {"st": st, "r0": st * STB}
                # psb (1-bank tiles), release order: pT (copies), dm (m1
                # stt), dh1/dh2/dh3 (h stts)
                A["pT"] = psb.tile([112, F], dt.bfloat16, tag="psb", name="pT")
                for nm in ("dm", "dh1", "dh2", "dh3"):
                    A[nm] = [psb.tile([80, 512], dt.float32, tag="psb", name=nm)
                             for _ in range(NCHUNK)]
                # psA (2-bank tiles) in release order (exp frees the psum)
                for nm in ("x1", "xs1", "x2", "s1", "xs2", "x3", "s2", "xs3", "s3"):
                    A[nm] = ps.tile([114, F], dt.float32, tag="ps", name=nm)
                A["out"] = ps.tile([65, F], dt.float32, tag="ps", name="pout")
                return A

            def mm(psum_t, wname, rhs_t, start, stop):
                for c in range(NCHUNK):
                    mm_c(psum_t, wname, rhs_t, start, stop, c)

            def mm_c(psum_t, wname, rhs_t, start, stop, c):
                ks, ms = w_spans(wname)
                off = W_OFF[wname]
                cs = slice(512 * c, 512 * (c + 1))
                nc.tensor.matmul(psum_t[0:ms, cs], wt[0:ks, off:off + ms],
                                 rhs_t[0:ks, cs], start=start, stop=stop)

            def softplus(psum_t, rows, bias_name, out_dtype, pool):
                stg = stgpool.tile([rows, F], dt.float32)
                nc.scalar.activation(stg[0:rows, :], psum_t[0:rows, :], AF.Exp,
                                     bias=bt[0:rows, B_COL[bias_name]:B_COL[bias_name] + 1])
                res = pool.tile([rows, F], out_dtype)
                nc.scalar.activation(res[0:rows, :], stg[0:rows, :], AF.Ln, bias=1.0)
                return res

            def dmm(tiles, wname, rhs_t):
                ks, ms = w_spans(wname)
                off = W_OFF[wname]
                for cc in range(NCHUNK):
                    cs = slice(512 * cc, 512 * (cc + 1))
                    nc.tensor.matmul(tiles[cc][0:ms, 0:512], wt[0:ks, off:off + ms],
                                     rhs_t[0:ks, cs], start=True, stop=True)

            def gate_tail(A, xs_p, cl_w, cp_w, bcl, dh_t, axs, asv):
                x0b = A["x0b"]
                g = gpool.tile([114, F], dt.bfloat16, tag="g")
                for cc in range(NCHUNK):
                    cs = slice(512 * cc, 512 * (cc + 1))
                    h = mhpool.tile([80, 512], dt.bfloat16, tag="mh")
                    nc.vector.scalar_tensor_tensor(
                        h[0:80, :], dh_t[cc][0:80, :],
                        bt[0:80, B_COL[bcl]:B_COL[bcl] + 1],
                        x0b[0:80, cs], op0=ALU.add, op1=ALU.mult)
                    ks, ms = w_spans(cl_w)
                    off = W_OFF[cl_w]
                    nc.tensor.matmul(xs_p[0:ms, cs], wt[0:ks, off:off + ms],
                                     h[0:ks, :], start=False, stop=False)
                for cc in range(NCHUNK):
                    cs = slice(512 * cc, 512 * (cc + 1))
                    nc.vector.tensor_tensor(g[0:114, cs], axs[0:114, cs],
                                            asv[0:114, cs], op=ALU.mult)
                    mm_c(xs_p, cp_w, g, False, cc == NCHUNK - 1, cc)

            def body(A):
                r0 = A["r0"]
                # ---- input load ----
                # in_t bf16 [128, NA*112]; block a (112 wide):
                # cols 112a+{0:16 x0s-PG0, 32:48 x0-PG0, 64:80 x0s-PG1, 96:112 x0-PG1}.
                in_t = inpool.tile([128, NA * 112], dt.bfloat16, tag="int")
                for pg in range(2):
                    rb = r0 + pg * F
                    src_x = inp_d.ap()[rb:rb + F, 0:16].rearrange("(a p) f -> p a f", p=128)
                    src_s = inp_d.ap()[rb:rb + F, 16:32].rearrange("(a p) f -> p a f", p=128)
                    r3 = in_t[:].rearrange("p (a q) -> p a q", q=112)
                    nc.sync.dma_start(out=r3[:, :, 64 * pg + 32:64 * pg + 48], in_=src_x)
                    nc.sync.dma_start(out=r3[:, :, 64 * pg:64 * pg + 16], in_=src_s)
                pT = A["pT"]
                for a in range(NA):
                    nc.tensor.transpose(pT[0:112, 128 * a:128 * (a + 1)],
                                        in_t[:, 112 * a:112 * a + 112], ct[:])
                # x0b rows: 0:16 x0s-PG0, 32:48 x0-PG0, 64:80 x0s-PG1,
                # 96:112 x0-PG1 (rows 0:80 double as the x0s operand)
                x0b = x0pool.tile([112, F], dt.bfloat16, tag="x0b")
                for cc in range(NCHUNK):
                    cs = slice(512 * cc, 512 * (cc + 1))
                    nc.vector.tensor_copy(x0b[0:112, cs], pT[0:112, cs])
                A["x0b"] = x0b

                # ---- L1 ----
                p_x1 = A["x1"]
                mm(p_x1, "W_xin", x0b, True, True)
                A["a_x1"] = softplus(p_x1, 114, "b_xin", dt.bfloat16, xpool)
                p_xs1 = A["xs1"]
                mm(p_xs1, "W_xl1", x0b, True, False)
                dmm(A["dm"], "W_clinm", x0b)
                ks, ms = w_spans("W_clin")
                off = W_OFF["W_clin"]
                for cc in range(NCHUNK):
                    cs = slice(512 * cc, 512 * (cc + 1))
                    m1 = mhpool.tile([80, 512], dt.bfloat16, tag="mh")
                    nc.vector.scalar_tensor_tensor(
                        m1[0:80, :], A["dm"][cc][0:80, :],
                        bt[0:80, B_COL["b_clinm"]:B_COL["b_clinm"] + 1],
                        x0b[0:80, cs], op0=ALU.add, op1=ALU.mult)
                    nc.tensor.matmul(p_xs1[0:ms, cs], wt[0:ks, off:off + ms],
                                     m1[0:ks, :], start=False, stop=cc == NCHUNK - 1)
                A["a_xs1"] = softplus(p_xs1, 114, "b_xl1", dt.bfloat16, xspool)

            def back2(A):
                a_x1 = A["a_x1"]
                # ---- L2 ----
                p_x2 = A["x2"]
                mm(p_x2, "W_xp1", a_x1, True, True)
                A["a_x2"] = softplus(p_x2, 114, "b_xp1", dt.bfloat16, xpool)
                p_s1 = A["s1"]
                mm(p_s1, "W_cp1m", a_x1, True, True)
                a_s1 = softplus(p_s1, 114, "b_cp1m", dt.bfloat16, xspool)
                p_xs2 = A["xs2"]
                mm(p_xs2, "W_xl2", a_x1, True, False)
                dmm(A["dh1"], "W_cl1m", a_x1)
                gate_tail(A, p_xs2, "W_cl1", "W_cp1", "b_cl1m", A["dh1"],
                          A["a_xs1"], a_s1)
                A["a_xs2"] = softplus(p_xs2, 114, "b_xl2", dt.bfloat16, xspool)

            def back34(A):
                a_x2, a_xs2 = A["a_x2"], A["a_xs2"]
                # ---- L3 ----
                p_x3 = A["x3"]
                mm(p_x3, "W_xp2", a_x2, True, True)
                a_x3 = softplus(p_x3, 114, "b_xp2", dt.bfloat16, xpool)
                p_s2 = A["s2"]
                mm(p_s2, "W_cp2m", a_x2, True, True)
                a_s2 = softplus(p_s2, 114, "b_cp2m", dt.bfloat16, xspool)
                p_xs3 = A["xs3"]
                mm(p_xs3, "W_xl3", a_x2, True, False)
                dmm(A["dh2"], "W_cl2m", a_x2)
                gate_tail(A, p_xs3, "W_cl2", "W_cp2", "b_cl2m", A["dh2"], a_xs2, a_s2)
                a_xs3 = softplus(p_xs3, 114, "b_xl3", dt.bfloat16, xspool)

                # ---- L4 / output ----
                p_s3 = A["s3"]
                mm(p_s3, "W_cpom", a_x3, True, True)
                a_s3 = softplus(p_s3, 114, "b_cpom", dt.bfloat16, xspool)
                p_out = A["out"]
                mm(p_out, "W_xlo", a_x3, True, False)
                dmm(A["dh3"], "W_clom", a_x3)
                gate_tail(A, p_out, "W_clo", "W_cpo", "b_clom", A["dh3"], a_xs3, a_s3)

            def tail(A):
                st, p_out = A["st"], A["out"]
                zc = zcpool.tile([65, F], dt.float32, tag="zc")
                nc.vector.tensor_copy(zc[0:65, :], p_out[0:65, :])
                for pg in range(2):
                    nc.sync.dma_start(out=zout[2 * st + pg:2 * st + pg + 1, :],
                                      in_=zc[64 * pg:64 * pg + 1, :])

            pending = None
            pending2 = None
            for st in range(nst):
                A = alloc_st(st)
                if pending is not None:
                    back2(pending)
                body(A)
                if pending2 is not None:
                    tail(pending2)
                    pending2 = None
                if pending is not None:
                    back34(pending)
                    pending2 = pending
                pending = A
            back2(pending)
            tail(pending2)
            back34(pending)
            tail(pending)

            # final softplus over the packed pre-activations + store
            rows = 2 * nst
            stg = stgpool.tile([rows, F], dt.float32)
            nc.scalar.activation(stg[0:rows, :], zout[0:rows, :], AF.Exp,
                                 bias=bt[0:rows, BXLO_ALL:BXLO_ALL + 1])
            a_fin = stgpool.tile([rows, F], dt.float32)
            nc.scalar.activation(a_fin[0:rows, :], stg[0:rows, :], AF.Ln, bias=1.0)
            dst = out_d.ap().rearrange("(p c) one -> p (c one)", p=rows)
            nc.sync.dma_start(out=dst, in_=a_fin[0:rows, :])

    nc.finalize()
    return nc



# ---------------------------------------------------------------------------
# Harness entry point: kernel(**inputs) takes the FULL (unsharded) inputs and
# returns the FULL [B, 1] float32 output. Internally shards the batch across
# the 8 NeuronCores (pure data parallel; weights replicated).
# ---------------------------------------------------------------------------
N_CORES = 8
_program_cache = {}


def _get_program(core_rows):
    if core_rows not in _program_cache:
        _program_cache[core_rows] = build_program(core_rows)
    return _program_cache[core_rows]


def kernel(**inputs):
    from concourse.bass_utils import run_bass_kernel_spmd
    x = np.ascontiguousarray(
        np.asarray(inputs["input"], dtype=np.float32).astype(ml_dtypes.bfloat16))
    B = x.shape[0]
    assert x.shape[1] == 2 * D
    core_rows = B // N_CORES
    assert core_rows * N_CORES == B and core_rows % STB == 0, (B,)
    wpack, bpack, ident = pack_weights(inputs)
    nc = _get_program(core_rows)
    in_maps = [{
        "input": x[i * core_rows:(i + 1) * core_rows],
        "wpack": wpack, "bpack": bpack, "ident": ident,
    } for i in range(N_CORES)]
    res = run_bass_kernel_spmd(nc, in_maps, list(range(N_CORES)))
    return np.concatenate([res.results[i]["out"] for i in range(N_CORES)], axis=0)


# revision 9
# speedup vs baseline: 1.0047x; 1.0047x over previous
# Dissipation network Bass kernel for TRN2 (bf16 matmuls, PG-fused block-diag).
#
# Layout: each "super-tile" (ST) covers 2*F batch rows = 2 partition groups
# (PG0 -> partitions 0:50 / 0:16, PG1 -> 64:114 / 64:80) x F free columns.
# Activations are stored transposed: [features, batch_cols].
# Each weight is packed BLOCK-DIAGONALLY: PG0 copy at rows rb:rb+k, cols
# 0:m and PG1 copy at rows 64+rb:.., cols 64:64+m, so ONE matmul
# instruction computes both partition groups.
# PSUM tiles for each super-tile are allocated UPFRONT in dependency
# order (x1, xs1, x2, s1, xs2, x3, s2, xs3, s3, out) so the pool's
# round-robin rotation (bufs=3) pairs each tile with one that releases
# early - avoiding false WAR serialization of the L3/L4 matmuls behind
# the slow xs gate chains.
# The final [B,1] output is NOT softplus'd per super-tile: the two psum
# rows are copied to SBUF, DMA-packed into a [2*nst, F] tile and a
# single softplus runs at the end (saves ~60us of ACT time).
# Input is converted to bf16 on the host: halves input DMA, transposes
# run at 1 cycle/row, and the transpose psum tile fits one PSUM bank.
# Softplus = Exp (with bias) then Ln(t + 1); both pinned to the
# natural_log_exp_and_others ACT table set (single table load).
import numpy as np
import ml_dtypes
import concourse.bass as bass
from concourse import bacc
import concourse.hw_specs as hw_specs
import concourse.bacc as bacc_mod
import concourse.mybir as mybir
import concourse.tile as tile

dt = mybir.dt
AF = mybir.ActivationFunctionType
ALU = mybir.AluOpType

_orig_get_tables = hw_specs.get_activation_tables


def _pinned_tables(arch):
    t = _orig_get_tables(arch)
    out = {}
    for name, fns in t.items():
        if name != "natural_log_exp_and_others":
            fns = fns - {AF.Exp, AF.Ln}
        out[name] = fns
    return out


bacc_mod.get_activation_tables = _pinned_tables

D, H = 16, 50
F = 1024            # free columns per PG block
STB = 2 * F         # rows per super-tile
NCHUNK = F // 512   # 512-col matmul chunks per F
NA = F // 128       # 128-col transpose chunks per F

W_SPECS = [
    ("W_xl1", D, H), ("W_xin", D, H), ("W_clinm", D, D), ("W_clin", D, H),
    ("W_xl2", H, H), ("W_cp1m", H, H), ("W_cl1m", H, D), ("W_xp1", H, H),
    ("W_cp1", H, H), ("W_cl1", D, H),
    ("W_xl3", H, H), ("W_cp2m", H, H), ("W_cl2m", H, D), ("W_xp2", H, H),
    ("W_cp2", H, H), ("W_cl2", D, H),
    ("W_xlo", H, 1), ("W_cpom", H, H), ("W_clom", H, D),
    ("W_cpo", H, 1), ("W_clo", D, 1),
]
W_KM = {n: (k, m) for n, k, m in W_SPECS}
X0_WEIGHTS = {"W_xl1", "W_xin", "W_clinm"}
W_OFF = {}
_off = 0
for _n, _k, _m in W_SPECS:
    W_OFF[_n] = _off
    _off += 64 + _m     # block-diag span: PG0 cols 0:m, PG1 cols 64:64+m
NW = _off


def w_spans(name):
    """(K_span, M_span) for the fused block-diagonal matmul."""
    k, m = W_KM[name]
    rb = 32 if name in X0_WEIGHTS else 0
    return 64 + rb + k, 64 + m


B_SPECS = ["b_xl1", "b_xin", "b_clinm", "b_xl2", "b_cp1m", "b_cl1m", "b_xp1",
           "b_xl3", "b_cp2m", "b_cl2m", "b_xp2", "b_xlo", "b_cpom", "b_clom"]
B_COL = {n: i for i, n in enumerate(B_SPECS)}
NB = len(B_SPECS) + 1
BXLO_ALL = NB - 1   # b_xlo replicated to all partitions (final staged softplus)


def pack_weights(inputs):
    wpack = np.zeros((128, NW), dtype=ml_dtypes.bfloat16)
    for n, k, m in W_SPECS:
        wt = np.asarray(inputs[n]).astype(np.float32).T  # [K, M]
        assert wt.shape == (k, m), (n, wt.shape)
        wb = wt.astype(ml_dtypes.bfloat16)
        rb = 32 if n in X0_WEIGHTS else 0
        o = W_OFF[n]
        wpack[rb:rb + k, o:o + m] = wb
        wpack[64 + rb:64 + rb + k, o + 64:o + 64 + m] = wb
    bpack = np.zeros((128, NB), dtype=np.float32)
    for n in B_SPECS:
        b = np.asarray(inputs[n]).astype(np.float32)
        c = B_COL[n]
        bpack[0:len(b), c] = b
        bpack[64:64 + len(b), c] = b
    bpack[:, BXLO_ALL] = float(np.asarray(inputs["b_xlo"]).astype(np.float32)[0])
    ident = np.eye(128, dtype=ml_dtypes.bfloat16)
    return wpack, bpack, ident


def build_program(n_rows):
    assert n_rows % STB == 0
    nst = n_rows // STB
    assert nst <= 64
    nc = bacc.Bacc("TRN2", target_bir_lowering=False, debug=False,
                   enable_asserts=False)
    inp_d = nc.dram_tensor("input", [n_rows, 32], dt.bfloat16, kind="ExternalInput")
    w_d = nc.dram_tensor("wpack", [128, NW], dt.bfloat16, kind="ExternalInput")
    b_d = nc.dram_tensor("bpack", [128, NB], dt.float32, kind="ExternalInput")
    c_d = nc.dram_tensor("ident", [128, 128], dt.bfloat16, kind="ExternalInput")
    out_d = nc.dram_tensor("out", [n_rows, 1], dt.float32, kind="ExternalOutput")

    with tile.TileContext(nc) as tc:
        with tc.tile_pool(name="const", bufs=1) as cpool, \
             tc.tile_pool(name="inp", bufs=4) as inpool, \
             tc.tile_pool(name="x0p", bufs=4) as x0pool, \
             tc.tile_pool(name="mh", bufs=6) as mhpool, \
             tc.tile_pool(name="g", bufs=4) as gpool, \
             tc.tile_pool(name="stg", bufs=8) as stgpool, \
             tc.tile_pool(name="axs", bufs=8) as xspool, \
             tc.tile_pool(name="ax", bufs=4) as xpool, \
             tc.tile_pool(name="zc", bufs=3) as zcpool, \
             tc.tile_pool(name="ps", bufs=3, space="PSUM") as ps, \
             tc.tile_pool(name="psb", bufs=2, space="PSUM") as psb:

            wt = cpool.tile([128, NW], dt.bfloat16, tag="wt")
            nc.sync.dma_start(out=wt[:], in_=w_d.ap())
            bt = cpool.tile([128, NB], dt.float32, tag="bt")
            nc.sync.dma_start(out=bt[:], in_=b_d.ap())
            ct = cpool.tile([128, 128], dt.bfloat16, tag="ct")
            nc.sync.dma_start(out=ct[:], in_=c_d.ap())
            # packed final-layer pre-activations: partition 2*st+pg holds
            # z_out for that PG's F batch rows
            zout = cpool.tile([2 * nst, F], dt.float32, tag="zout")

            # Zero the in_t staging buffers once: fused matmuls read the
            # full partition span (junk rows hit zero weights; must not
            # be NaN).
            for _ in range(4):
                z = inpool.tile([128, NA * 112], dt.bfloat16, tag="int")
                nc.vector.memset(z[:], 0.0)

            def alloc_st(st):
                A = {"st": st, "r0": st * STB}
                # psb (1-bank tiles), release order: pT (copies), dm (m1
                # stt), dh1/dh2/dh3 (h stts)
                A["pT"] = psb.tile([112, F], dt.bfloat16, tag="psb", name="pT")
                for nm in ("dm", "dh1", "dh2", "dh3"):
                    A[nm] = [psb.tile([80, 512], dt.float32, tag="psb", name=nm)
                             for _ in range(NCHUNK)]
                # psA (2-bank tiles) in release order (exp frees the psum)
                for nm in ("x1", "xs1", "x2", "s1", "xs2", "x3", "s2", "xs3", "s3"):
                    A[nm] = ps.tile([114, F], dt.float32, tag="ps", name=nm)
                A["out"] = ps.tile([65, F], dt.float32, tag="ps", name="pout")
                return A

            def mm(psum_t, wname, rhs_t, start, stop):
                for c in range(NCHUNK):
                    mm_c(psum_t, wname, rhs_t, start, stop, c)

            def mm_c(psum_t, wname, rhs_t, start, stop, c):
                ks, ms = w_spans(wname)
                off = W_OFF[wname]
                cs = slice(512 * c, 512 * (c + 1))
                nc.tensor.matmul(psum_t[0:ms, cs], wt[0:ks, off:off + ms],
                                 rhs_t[0:ks, cs], start=start, stop=stop)

            def softplus(psum_t, rows, bias_name, out_dtype, pool):
                stg = stgpool.tile([rows, F], dt.float32)
                nc.scalar.activation(stg[0:rows, :], psum_t[0:rows, :], AF.Exp,
                                     bias=bt[0:rows, B_COL[bias_name]:B_COL[bias_name] + 1])
                res = pool.tile([rows, F], out_dtype)
                nc.scalar.activation(res[0:rows, :], stg[0:rows, :], AF.Ln, bias=1.0)
                return res

            def dmm(tiles, wname, rhs_t):
                ks, ms = w_spans(wname)
                off = W_OFF[wname]
                for cc in range(NCHUNK):
                    cs = slice(512 * cc, 512 * (cc + 1))
                    nc.tensor.matmul(tiles[cc][0:ms, 0:512], wt[0:ks, off:off + ms],
                                     rhs_t[0:ks, cs], start=True, stop=True)

            def gate_tail(A, xs_p, cl_w, cp_w, bcl, dh_t, axs, asv):
                x0b = A["x0b"]
                g = gpool.tile([114, F], dt.bfloat16, tag="g")
                for cc in range(NCHUNK):
                    cs = slice(512 * cc, 512 * (cc + 1))
                    h = mhpool.tile([80, 512], dt.bfloat16, tag="mh")
                    nc.vector.scalar_tensor_tensor(
                        h[0:80, :], dh_t[cc][0:80, :],
                        bt[0:80, B_COL[bcl]:B_COL[bcl] + 1],
                        x0b[0:80, cs], op0=ALU.add, op1=ALU.mult)
                    ks, ms = w_spans(cl_w)
                    off = W_OFF[cl_w]
                    nc.tensor.matmul(xs_p[0:ms, cs], wt[0:ks, off:off + ms],
                                     h[0:ks, :], start=False, stop=False)
                for cc in range(NCHUNK):
                    cs = slice(512 * cc, 512 * (cc + 1))
                    nc.vector.tensor_tensor(g[0:114, cs], axs[0:114, cs],
                                            asv[0:114, cs], op=ALU.mult)
                    mm_c(xs_p, cp_w, g, False, cc == NCHUNK - 1, cc)

            def body(A):
                r0 = A["r0"]
                # ---- input load ----
                # in_t bf16 [128, NA*112]; block a (112 wide):
                # cols 112a+{0:16 x0s-PG0, 32:48 x0-PG0, 64:80 x0s-PG1, 96:112 x0-PG1}.
                in_t = inpool.tile([128, NA * 112], dt.bfloat16, tag="int")
                for pg in range(2):
                    rb = r0 + pg * F
                    src_x = inp_d.ap()[rb:rb + F, 0:16].rearrange("(a p) f -> p a f", p=128)
                    src_s = inp_d.ap()[rb:rb + F, 16:32].rearrange("(a p) f -> p a f", p=128)
                    r3 = in_t[:].rearrange("p (a q) -> p a q", q=112)
                    nc.sync.dma_start(out=r3[:, :, 64 * pg + 32:64 * pg + 48], in_=src_x)
                    nc.sync.dma_start(out=r3[:, :, 64 * pg:64 * pg + 16], in_=src_s)
                pT = A["pT"]
                for a in range(NA):
                    nc.tensor.transpose(pT[0:112, 128 * a:128 * (a + 1)],
                                        in_t[:, 112 * a:112 * a + 112], ct[:])
                # x0b rows: 0:16 x0s-PG0, 32:48 x0-PG0, 64:80 x0s-PG1,
                # 96:112 x0-PG1 (rows 0:80 double as the x0s operand)
                x0b = x0pool.tile([112, F], dt.bfloat16, tag="x0b")
                nc.vector.tensor_copy(x0b[0:112, :], pT[0:112, :])
                A["x0b"] = x0b

                # ---- L1 ----
                p_x1 = A["x1"]
                mm(p_x1, "W_xin", x0b, True, True)
                A["a_x1"] = softplus(p_x1, 114, "b_xin", dt.bfloat16, xpool)
                p_xs1 = A["xs1"]
                mm(p_xs1, "W_xl1", x0b, True, False)
                dmm(A["dm"], "W_clinm", x0b)
                ks, ms = w_spans("W_clin")
                off = W_OFF["W_clin"]
                for cc in range(NCHUNK):
                    cs = slice(512 * cc, 512 * (cc + 1))
                    m1 = mhpool.tile([80, 512], dt.bfloat16, tag="mh")
                    nc.vector.scalar_tensor_tensor(
                        m1[0:80, :], A["dm"][cc][0:80, :],
                        bt[0:80, B_COL["b_clinm"]:B_COL["b_clinm"] + 1],
                        x0b[0:80, cs], op0=ALU.add, op1=ALU.mult)
                    nc.tensor.matmul(p_xs1[0:ms, cs], wt[0:ks, off:off + ms],
                                     m1[0:ks, :], start=False, stop=cc == NCHUNK - 1)
                A["a_xs1"] = softplus(p_xs1, 114, "b_xl1", dt.bfloat16, xspool)

            def back2(A):
                a_x1 = A["a_x1"]
                # ---- L2 ----
                p_x2 = A["x2"]
                mm(p_x2, "W_xp1", a_x1, True, True)
                A["a_x2"] = softplus(p_x2, 114, "b_xp1", dt.bfloat16, xpool)
                p_s1 = A["s1"]
                mm(p_s1, "W_cp1m", a_x1, True, True)
                a_s1 = softplus(p_s1, 114, "b_cp1m", dt.bfloat16, xspool)
                p_xs2 = A["xs2"]
                mm(p_xs2, "W_xl2", a_x1, True, False)
                dmm(A["dh1"], "W_cl1m", a_x1)
                gate_tail(A, p_xs2, "W_cl1", "W_cp1", "b_cl1m", A["dh1"],
                          A["a_xs1"], a_s1)
                A["a_xs2"] = softplus(p_xs2, 114, "b_xl2", dt.bfloat16, xspool)

            def back34(A):
                a_x2, a_xs2 = A["a_x2"], A["a_xs2"]
                # ---- L3 ----
                p_x3 = A["x3"]
                mm(p_x3, "W_xp2", a_x2, True, True)
                a_x3 = softplus(p_x3, 114, "b_xp2", dt.bfloat16, xpool)
                p_s2 = A["s2"]
                mm(p_s2, "W_cp2m", a_x2, True, True)
                a_s2 = softplus(p_s2, 114, "b_cp2m", dt.bfloat16, xspool)
                p_xs3 = A["xs3"]
                mm(p_xs3, "W_xl3", a_x2, True, False)
                dmm(A["dh2"], "W_cl2m", a_x2)
                gate_tail(A, p_xs3, "W_cl2", "W_cp2", "b_cl2m", A["dh2"], a_xs2, a_s2)
                a_xs3 = softplus(p_xs3, 114, "b_xl3", dt.bfloat16, xspool)

                # ---- L4 / output ----
                p_s3 = A["s3"]
                mm(p_s3, "W_cpom", a_x3, True, True)
                a_s3 = softplus(p_s3, 114, "b_cpom", dt.bfloat16, xspool)
                p_out = A["out"]
                mm(p_out, "W_xlo", a_x3, True, False)
                dmm(A["dh3"], "W_clom", a_x3)
                gate_tail(A, p_out, "W_clo", "W_cpo", "b_clom", A["dh3"], a_xs3, a_s3)

            def tail(A):
                st, p_out = A["st"], A["out"]
                zc = zcpool.tile([65, F], dt.float32, tag="zc")
                nc.vector.tensor_copy(zc[0:65, :], p_out[0:65, :])
                for pg in range(2):
                    nc.sync.dma_start(out=zout[2 * st + pg:2 * st + pg + 1, :],
                                      in_=zc[64 * pg:64 * pg + 1, :])

            pending = None
            pending2 = None
            for st in range(nst):
                A = alloc_st(st)
                if pending is not None:
                    back2(pending)
                body(A)
                if pending2 is not None:
                    tail(pending2)
                    pending2 = None
                if pending is not None:
                    back34(pending)
                    pending2 = pending
                pending = A
            back2(pending)
            tail(pending2)
            back34(pending)
            tail(pending)

            # final softplus over the packed pre-activations + store
            rows = 2 * nst
            stg = stgpool.tile([rows, F], dt.float32)
            nc.scalar.activation(stg[0:rows, :], zout[0:rows, :], AF.Exp,
                                 bias=bt[0:rows, BXLO_ALL:BXLO_ALL + 1])
            a_fin = stgpool.tile([rows, F], dt.float32)
            nc.scalar.activation(a_fin[0:rows, :], stg[0:rows, :], AF.Ln, bias=1.0)
            dst = out_d.ap().rearrange("(p c) one -> p (c one)", p=rows)
            nc.sync.dma_start(out=dst, in_=a_fin[0:rows, :])

    nc.finalize()
    return nc



# ---------------------------------------------------------------------------
# Harness entry point: kernel(**inputs) takes the FULL (unsharded) inputs and
# returns the FULL [B, 1] float32 output. Internally shards the batch across
# the 8 NeuronCores (pure data parallel; weights replicated).
# ---------------------------------------------------------------------------
N_CORES = 8
_program_cache = {}


def _get_program(core_rows):
    if core_rows not in _program_cache:
        _program_cache[core_rows] = build_program(core_rows)
    return _program_cache[core_rows]


def kernel(**inputs):
    from concourse.bass_utils import run_bass_kernel_spmd
    x = np.ascontiguousarray(
        np.asarray(inputs["input"], dtype=np.float32).astype(ml_dtypes.bfloat16))
    B = x.shape[0]
    assert x.shape[1] == 2 * D
    core_rows = B // N_CORES
    assert core_rows * N_CORES == B and core_rows % STB == 0, (B,)
    wpack, bpack, ident = pack_weights(inputs)
    nc = _get_program(core_rows)
    in_maps = [{
        "input": x[i * core_rows:(i + 1) * core_rows],
        "wpack": wpack, "bpack": bpack, "ident": ident,
    } for i in range(N_CORES)]
    res = run_bass_kernel_spmd(nc, in_maps, list(range(N_CORES)))
    return np.concatenate([res.results[i]["out"] for i in range(N_CORES)], axis=0)
